# revision 27
# baseline (speedup 1.0000x reference)
"""Trainium2 Bass kernel for nn_CustomAttention (B=8, S=1024, H=1024, NH=16).

Strategy: data-parallel over batch — one batch element per NeuronCore, no
collectives. Host does layout-only prep (transposes / fp8+bf16 casts); all
FLOPs run on device.

v3: the Q/K/V projections run as fp8e4m3 DoubleRow matmuls (2 output
cols/cycle) with a 3-term RESIDUAL decomposition that keeps full accuracy:
  A  = fp8(32*W.T)              (weights in e4m3's sweet spot)
  B  = fp8(1024*(W.T - A/32))   (weight quantization residual)
  h8 = fp8(hs.T), d8 = fp8(32*(hs.T - h8))   (activation + residual)
  psA = h8@A + d8@A   (scale 32*q)     psB = h8@B   (scale 1024*q-residual)
  qt  = psA + psB/32 + 32*bq   (DVE: scalar_tensor_tensor + tensor_scalar_add)
Dropped terms are O(eps^2) ~ 0.1%, so logit noise stays ~0.002 (budget 0.015
— plain fp8's 0.04 logit noise fails the 2e-2 gate on concentrated softmax
rows). Projections cost 12 DR c-steps per 512-col chunk = 0.75x of bf16.
Scores / exp / ctx stay bf16 (exact-enough): fp8 anywhere in the softmax
value path was measured over-tolerance.

Scale bookkeeping: qt/kt = 32*(q+bq) bf16 -> scores psum = 1024*logits*8;
exp scale = (1/8)/1024; V' = 32*V bf16 with ones-col 32.0 so the DVE
reciprocal of den = 1/(32*Z) cancels everything in the normalize.

Per-core dataflow (as v1 otherwise):
  scoresT[s,l] per head = kt_h(stationary) . qt_h -> PSUM [128, S]
  expT = Exp(scores * scale) -> bf16 SBUF
  ctx[l,d] per (head, l): stationary expT chunk, moving V'[s, 65] (64 V cols
    + ones col) accumulated over s-tiles in one PSUM bank.
  normalize: DVE reciprocal + scalar_tensor_tensor (folds bv).
Schedule: software-pipelined sections; section t runs scores+exp(t), a ctx
work-queue (quota-paced so PE never outruns Act), proj(t+1) chunk-steps, and
V' chunk-steps in sections 0-2. PE is the critical engine (~144us busy);
Act's exp stream is ~133us; all steady-state DMAs stay off the Act queue.
PSUM: scores 2x2 banks, projA 1, projB 1, ctx 2x1 = 8 banks.
"""
import sys

sys.path.insert(0, "/opt/trn_rl_repo")

import numpy as np
import ml_dtypes
from collections import deque
from contextlib import ExitStack

from concourse import bacc, tile, mybir
from concourse.bass_utils import run_bass_kernel_spmd

F32 = mybir.dt.float32
BF16 = mybir.dt.bfloat16
FP8 = mybir.dt.float8e4
AF = mybir.ActivationFunctionType
ALU = mybir.AluOpType
DR = mybir.MatmulPerfMode.DoubleRow

P = 128
HD = 64
N_CORES = 8
WSCALE = 32.0
RESCALE = 1024.0  # weight-residual scale (WSCALE * WSCALE)
RINV = 1.0 / WSCALE

FP8NP = ml_dtypes.float8_e4m3


def _chunks(total, size=512):
    out = []
    a = 0
    while a < total:
        out.append((a, min(a + size, total)))
        a += size
    return out


def build_program(S, H, NH, num_devices=N_CORES):
    """One SPMD program; every core runs it on its own batch element."""
    KT = H // P          # h-tiles (contraction tiles)
    KP = KT // 2         # DoubleRow contraction pairs
    NT = H // P          # o-tiles
    ST = S // P          # s-tiles / l-tiles
    HPT = P // HD        # heads per o-tile (2)
    assert NH * HD == H and HPT == 2 and S == H
    SCALE = 1.0 / float(np.sqrt(HD))
    EXP_SCALE = SCALE / (WSCALE * WSCALE)

    nc = bacc.Bacc(
        "TRN2", target_bir_lowering=False, debug=False, num_devices=num_devices
    )

    # hsT pairs: row (c*P+p) = concat_i X[(2c+i)*P+p, :]
    h8p = nc.dram_tensor("h8p", [KP * P, 2 * S], FP8, kind="ExternalInput")
    d8p = nc.dram_tensor("d8p", [KP * P, 2 * S], FP8, kind="ExternalInput")
    # q/k weights DoubleRow-packed and group-merged per o-tile row block:
    # row (t*P+p), col (g*KP*2*P + c*2P + i*P + m), g in {qA,qB,kA,kB}
    wqk = nc.dram_tensor("wqk", [NT * P, 4 * KP * 2 * P], FP8,
                         kind="ExternalInput")
    # wv pairs (moving operand layout), A|B merged: col (s*2H + i*H + o)
    wvab = nc.dram_tensor("wvab", [KP * P, 2 * 2 * H], FP8,
                          kind="ExternalInput")
    bqk = nc.dram_tensor("bqk", [P, 2 * NT], F32, kind="ExternalInput")
    bv_row = nc.dram_tensor("bv_row", [1, H], F32, kind="ExternalInput")
    outD = nc.dram_tensor("out", [S, H], F32, kind="ExternalOutput")

    with tile.TileContext(nc) as tc, ExitStack() as ctx:
        consts = ctx.enter_context(tc.tile_pool(name="consts", bufs=1))
        hstp = ctx.enter_context(tc.tile_pool(name="hstp", bufs=2))
        wstr = ctx.enter_context(tc.tile_pool(name="wstr", bufs=3))
        qkp = ctx.enter_context(tc.tile_pool(name="qkp", bufs=4))
        tmpp = ctx.enter_context(tc.tile_pool(name="tmpp", bufs=2))
        vvp = ctx.enter_context(tc.tile_pool(name="vvp", bufs=ST))
        expp = ctx.enter_context(tc.tile_pool(name="expp", bufs=54))
        rpl = ctx.enter_context(tc.tile_pool(name="rpl", bufs=4))
        outp = ctx.enter_context(tc.tile_pool(name="outp", bufs=2))
        big = ctx.enter_context(tc.tile_pool(name="big", bufs=2, space="PSUM"))
        prA = ctx.enter_context(tc.tile_pool(name="prA", bufs=1, space="PSUM"))
        prB = ctx.enter_context(tc.tile_pool(name="prB", bufs=1, space="PSUM"))
        cxp = ctx.enter_context(tc.tile_pool(name="cxp", bufs=2, space="PSUM"))

        # ---- input DMA: w(0) and hsT tiles first (feed the PE asap).
        # Few LARGE transfers: each dma_start costs ~0.6us of HWDGE
        # descriptor-gen, which was the v3.1 startup bottleneck.
        w_t = {}  # t -> [P, 4, KP, 2, P] tile; g in {qA,qB,kA,kB}

        def load_w(t):
            if t >= NT or t in w_t:
                return
            w = wstr.tile([P, 4, KP, 2, P], FP8, tag="wstr", name=f"w{t}")
            nc.sync.dma_start(
                out=w[:],
                in_=wqk[t * P : (t + 1) * P, :].rearrange(
                    "p (g c i m) -> p g c i m", g=4, c=KP, i=2
                ),
            )
            w_t[t] = w

        load_w(0)

        # h8 tiles on the gpsimd DMA queue (25ns issue cost); d8 on the SP
        # queue after the first weights (the proj pass order A,B,d8 gives d8
        # until ~6us to land). NOTHING rides the Act queue before the first
        # exp: each dma_start costs its sequencer ~667ns, directly delaying
        # Act's pipeline. wvB DMAs are emitted mid-section-0 between exps.
        h8all = hstp.tile([P, KP, 2, S], FP8, tag="ht", name="h8all")
        nc.gpsimd.dma_start(
            out=h8all[:],
            in_=h8p[:, :].rearrange("(c p) (i l) -> p c i l", p=P, i=2),
        )
        h8 = [h8all[:, c] for c in range(KP)]

        # q/k biases: one small DMA, needed first at the proj(0) drain
        consts_bqk = consts.tile([P, 2 * NT], F32, tag="bqk")
        nc.sync.dma_start(out=consts_bqk[:], in_=bqk[:])

        d8all = hstp.tile([P, KP, 2, S], FP8, tag="ht", name="d8all")
        nc.sync.dma_start(
            out=d8all[:],
            in_=d8p[:, :].rearrange("(c p) (i l) -> p c i l", p=P, i=2),
        )
        d8 = [d8all[:, c] for c in range(KP)]

        # wv moving tiles (A|B merged, one transfer on the idle gpsimd
        # queue; first needed at section 0 j>=3, lands ~9us)
        wvpool = ctx.enter_context(tc.tile_pool(name="wvpool", bufs=1))
        wvt = wvpool.tile([P, KP, 2, 2, H], FP8, tag="wv", name="wvab")
        nc.gpsimd.dma_start(
            out=wvt[:],
            in_=wvab[:, :].rearrange("(c p) (s i o) -> p c s i o", p=P, s=2, i=2),
        )
        wva = [wvt[:, c, 0] for c in range(KP)]
        wvb = [wvt[:, c, 1] for c in range(KP)]

        load_w(1)

        # ---- bv broadcast (first needed at the first ctx normalize) ----
        bv_sb = consts.tile([1, H], F32, tag="bv")
        bvb = consts.tile([P, H], F32, tag="bvb")
        nc.sync.dma_start(out=bv_sb[:], in_=bv_row[:])
        nc.gpsimd.partition_broadcast(bvb[:], bv_sb[:])

        qt_t = {}
        kt_t = {}
        ex_t = {}  # t -> {(hh, j): exp tile}
        vv = []

        # ---- emission helpers ----
        def proj_steps(t, kbig=False):
            """Generator yielding per 4-matmul pass (3 passes per 512-col
            chunk, 12 per matrix-tile): Q then K of o-tile t. kbig: K's
            psums live in one big-pool tile (preamble only, so K need not
            wait on Q's psum drain)."""
            for gi, (mat, bcol, store, tag) in enumerate(
                (("q", t, qt_t, "qt"), ("k", NT + t, kt_t, "kt"))
            ):
                wa = w_t[t][:, 2 * gi]
                wb = w_t[t][:, 2 * gi + 1]
                ot = qkp.tile([P, S], BF16, tag=tag, name=f"{tag}{t}")
                store[t] = ot
                for (a, b) in _chunks(S):
                    if kbig and mat == "k":
                        ps = big.tile([P, S], F32, tag="big", name=f"kps{a}")
                        psA_ap, psB_ap = ps[:, 0:512], ps[:, 512:1024]
                    else:
                        psA_ap = prA.tile([P, 512], F32, tag="prA",
                                          name="pA")[:]
                        psB_ap = prB.tile([P, 512], F32, tag="prB",
                                          name="pB")[:]
                    # psA = h8@A (scale 32*q); psB = d8@A + h8@B (both scale
                    # 1024: d8 and B each carry a 32x residual boost).
                    for c in range(KP):
                        nc.tensor.matmul(
                            psA_ap, wa[:, c, :, :], h8[c][:, :, a:b],
                            start=(c == 0), stop=(c == KP - 1), perf_mode=DR,
                        )
                    yield
                    for c in range(KP):
                        nc.tensor.matmul(
                            psB_ap, wb[:, c, :, :], h8[c][:, :, a:b],
                            start=(c == 0), stop=False, perf_mode=DR,
                        )
                    yield
                    for c in range(KP):
                        nc.tensor.matmul(
                            psB_ap, wa[:, c, :, :], d8[c][:, :, a:b],
                            start=False, stop=(c == KP - 1), perf_mode=DR,
                        )
                    # ISA s2s2d2: two PSUM tensor srcs are illegal, so
                    # combine in two one-PSUM ops.
                    tmp = tmpp.tile([P, 512], BF16, tag="tmp", name="tmp")
                    nc.vector.tensor_scalar(
                        tmp[:], psB_ap, RINV, consts_bqk[:, bcol : bcol + 1],
                        ALU.mult, ALU.add,
                    )
                    nc.vector.tensor_tensor(ot[:, a:b], psA_ap, tmp[:],
                                            ALU.add)
                    yield
            while True:
                yield

        def vprime_steps():
            """Generator yielding one atomic 512-col chunk (12 DR matmuls +
            drain) per step; psums ride a big-pool tile (split A|B halves)
            so the interleave with scores can never deadlock the prA/prB
            rings. V' = 32*V in bf16; col 64 = 32.0 (the moving ones-column
            that produces softmax denominators)."""
            for m in range(ST):
                vt = vvp.tile([P, NH, 66], BF16, tag="vv", name=f"vv{m}")
                for ci, (a, b) in enumerate(_chunks(H)):
                    psA = cxp.tile([P, 512], F32, tag="cx", name=f"vA{m}_{ci}")[:]
                    psB = cxp.tile([P, 512], F32, tag="cx", name=f"vB{m}_{ci}")[:]
                    for c in range(KP):
                        nc.tensor.matmul(
                            psA, h8[c][:, :, m * P : (m + 1) * P],
                            wva[c][:, :, a:b],
                            start=(c == 0), stop=(c == KP - 1), perf_mode=DR,
                        )
                    for c in range(KP):
                        nc.tensor.matmul(
                            psB, d8[c][:, :, m * P : (m + 1) * P],
                            wva[c][:, :, a:b],
                            start=(c == 0), stop=False, perf_mode=DR,
                        )
                    for c in range(KP):
                        nc.tensor.matmul(
                            psB, h8[c][:, :, m * P : (m + 1) * P],
                            wvb[c][:, :, a:b],
                            start=False, stop=(c == KP - 1), perf_mode=DR,
                        )
                    h0 = a // HD
                    tmpv = tmpp.tile([P, 512], BF16, tag="tmp", name="tmpv")
                    nc.vector.tensor_scalar_mul(tmpv[:], psB, RINV)
                    nc.vector.tensor_tensor(
                        vt[:, h0 : h0 + 512 // HD, 0:64],
                        psA.rearrange("p (h d) -> p h d", d=HD),
                        tmpv[:].rearrange("p (h d) -> p h d", d=HD),
                        ALU.add,
                    )
                    if ci == 1:
                        nc.vector.memset(vt[:, :, 64:65], WSCALE)
                        vv.append(vt)
                    yield
            while True:
                yield

        def emit_scores_exp(t, j):
            exs = ex_t.setdefault(t, {})
            for hh in range(HPT):
                r0 = hh * HD
                sc = big.tile([P, S], F32, tag="big", name=f"sc{t}_{j}_{hh}")
                for (a, b) in _chunks(S):
                    nc.tensor.matmul(
                        sc[:, a:b],
                        kt_t[t][r0 : r0 + HD, j * P : (j + 1) * P],
                        qt_t[t][r0 : r0 + HD, a:b],
                        start=True, stop=True,
                        tile_position=(r0, 0),
                    )
                e = expp.tile([P, S], BF16, tag="ex", name=f"ex{t}_{j}_{hh}")
                nc.scalar.activation(e[:], sc[:], AF.Exp, scale=EXP_SCALE)
                exs[(hh, j)] = e

        def emit_ctx(t, l, ot, cx_pool=None, cx_tag="cx"):
            exs = ex_t[t]
            pool_ = cx_pool or cxp
            cx = pool_.tile([P, 512], F32, tag=cx_tag, name=f"cx{t}_{l}")
            for hh in range(HPT):
                h = HPT * t + hh
                o = hh * 65
                for j in range(ST):
                    nc.tensor.matmul(
                        cx[:, o : o + 65],
                        exs[(hh, j)][:, l * P : (l + 1) * P],
                        vv[j][:, h, 0:65],
                        start=(j == 0), stop=(j == ST - 1),
                    )
            rc = rpl.tile([P, HPT], F32, tag="rc", name=f"rc{t}_{l}")
            dens = cx[:, 0 : 2 * 65].rearrange("p (h x) -> p h x", x=65)[:, :, 64:65]
            nc.vector.reciprocal(rc[:].rearrange("p (h x) -> p h x", x=1), dens)
            for hh in range(HPT):
                nc.vector.scalar_tensor_tensor(
                    ot[:, l, hh * HD : (hh + 1) * HD],
                    cx[:, hh * 65 : hh * 65 + 64],
                    rc[:, hh : hh + 1],
                    bvb[:, (HPT * t + hh) * HD : (HPT * t + hh + 1) * HD],
                    ALU.mult,
                    ALU.add,
                )

        # ---- PE warm-up ----
        # The PE clock ramps to full rate only after ~3us of continuous
        # execution; real work can't start until the first weight/hs DMAs
        # land, so burn that wait on scratch matmuls.
        wu = consts.tile([P, 3 * P], BF16, tag="wu")
        nc.vector.memset(wu[:], 0.0)
        wups = cxp.tile([P, 512], F32, tag="cx", name="wups")
        for i in range(12):
            nc.tensor.matmul(
                wups[:, 0:256], wu[:, 0:P], wu[:, P : P + 256],
                start=(i == 0), stop=(i == 11),
            )

        # preamble: proj(0) Q fully + K chunk0 (scores j<=3 only read kt
        # cols < 512, so K chunk1 smears into section 0). K's psums go to
        # the big pool so they need not wait on Q's chunk drains.
        p0 = proj_steps(0, kbig=True)
        for _ in range(9):
            next(p0)

        vgen = vprime_steps()
        nvp = 0  # V' pass-steps emitted so far

        ots = {}
        ctx_done = {}

        def ctx_unit(tc_, l, split_dma=False, cx_pool=None, cx_tag="cx"):
            """Emit one ctx+normalize unit; DMA the o-tile column when all
            ST units of tc_ have been emitted (split_dma: one DMA per l)."""
            if tc_ not in ots:
                ots[tc_] = outp.tile([P, ST, P], F32, tag="ou", name=f"ou{tc_}")
                ctx_done[tc_] = 0
            emit_ctx(tc_, l, ots[tc_], cx_pool=cx_pool, cx_tag=cx_tag)
            ctx_done[tc_] += 1
            if split_dma:
                if l % 2:  # DMA l-1..l as one transfer, alternating queues
                    eng = nc.gpsimd if l % 4 == 1 else nc.sync
                    eng.dma_start(
                        out=outD[
                            (l - 1) * P : (l + 1) * P,
                            tc_ * P : (tc_ + 1) * P,
                        ].rearrange("(l p) c -> p l c", p=P),
                        in_=ots[tc_][:, l - 1 : l + 1, :],
                    )
            elif ctx_done[tc_] == ST:
                nc.gpsimd.dma_start(
                    out=outD[:, tc_ * P : (tc_ + 1) * P].rearrange(
                        "(l p) c -> p l c", p=P
                    ),
                    in_=ots[tc_][:],
                )

        # ---- sections 0..NT-1 ----
        # scores/exp(t) pace Act; proj(t+1) smears 12 pass-steps/section;
        # V' smears its 48 pass-steps over sections 0-3; ctx units drain
        # from a quota-paced queue (tile t-1 units only at j>=3: its last
        # exps land ~2us into section t). Per-j PE work stays just above
        # Act's ~2.1us/j exp pace so neither engine stalls the psum ring.
        cqueue = deque()

        def vchunks(t, j):
            # 16 atomic V' chunks over sections 0-2
            if t == 0:
                return 1 if j >= 3 else 0
            if t == 1:
                return 1 if j <= 5 else 0
            if t == 2:
                return 1 if j <= 4 else 0
            return 0

        def cquota(t, j):
            # front-load each tile's units so its exp tiles free up before
            # the expp ring (bufs=56) wraps around to reuse them.
            if t == 2:
                return 1 if j >= 6 else 0
            if t == 3:
                return 2 if j <= 2 else 1
            if 4 <= t <= 6:
                return 1 + (1 if j >= 4 else 0)
            if t == 7:
                return 2
            return 0

        for t in range(NT):
            pgen = proj_steps(t + 1) if t + 1 < NT else None
            if t >= 1:
                cqueue.extend((t - 1, l) for l in range(ST))
            for j in range(ST):
                emit_scores_exp(t, j)
                if t == 0 and j == 0:
                    # K chunk1 of proj(0): must fully precede sc(0,1)'s big-
                    # pool alloc (its drain frees Kc1's slot in the ring).
                    for _ in range(3):
                        next(p0)
                for _ in range(vchunks(t, j)):
                    next(vgen)
                    nvp += 1
                npop = cquota(t, j)
                while npop > 0 and cqueue and (
                    cqueue[0][0] <= t - 2 or j >= 3
                ):
                    tc_, l = cqueue.popleft()
                    ctx_unit(tc_, l)
                    npop -= 1
                if pgen is not None:
                    next(pgen)
                    if j % 2 == 1:
                        next(pgen)
            load_w(t + 2)
        assert nvp == 2 * ST

        # tail: remaining ctx units (tile 7, plus any stragglers), rotating
        # the ctx PSUM through the now-idle proj/scores rings.
        cqueue.extend((NT - 1, l) for l in range(ST))
        tail_rot = [(cxp, "cx"), (prA, "prA"), (big, "big")]
        i = 0
        while cqueue:
            tc_, l = cqueue.popleft()
            pool_, tag_ = tail_rot[i % 3]
            i += 1
            ctx_unit(tc_, l, split_dma=(tc_ == NT - 1), cx_pool=pool_,
                     cx_tag=tag_)

    nc.compile()
    return nc


_CACHE = {}


def _get_program(S, H, NH, num_devices):
    key = (S, H, NH, num_devices)
    if key not in _CACHE:
        _CACHE[key] = build_program(S, H, NH, num_devices)
    return _CACHE[key]


def make_in_maps(hidden_states, Wq, bq, Wk, bk, Wv, bv):
    B, S, H = hidden_states.shape
    NT = H // P
    KT = H // P
    KP = KT // 2

    def pack_pair_rows(X):
        # [KT*P, W] -> [KP*P, 2W]: row c*P+p = concat_i X[(2c+i)*P+p, :]
        r = np.ascontiguousarray(X).reshape(KP, 2, P, -1)
        return np.ascontiguousarray(r.transpose(0, 2, 1, 3).reshape(KP * P, -1))

    def pack_w_dr(wT):
        # DoubleRow stationary pack: row (t*P+p), col (c*2P+i*P+m)
        #   = wT[(2c+i)*P+p, t*P+m]
        w5 = np.ascontiguousarray(wT).reshape(KP, 2, P, NT, P)
        return np.ascontiguousarray(
            w5.transpose(3, 2, 0, 1, 4).reshape(NT * P, KP * 2 * P)
        )

    def residual_pair(wT):
        A = (WSCALE * wT).astype(FP8NP)
        Bm = (RESCALE * (wT - A.astype(np.float32) / WSCALE)).astype(FP8NP)
        return A, Bm

    qA, qB = residual_pair(np.ascontiguousarray(Wq.T.astype(np.float32)))
    kA, kB = residual_pair(np.ascontiguousarray(Wk.T.astype(np.float32)))
    vA, vB = residual_pair(np.ascontiguousarray(Wv.T.astype(np.float32)))

    wqk_ = np.concatenate(
        [pack_w_dr(qA), pack_w_dr(qB), pack_w_dr(kA), pack_w_dr(kB)], axis=1
    )
    wvab_ = np.concatenate([pack_pair_rows(vA), pack_pair_rows(vB)], axis=1)

    bqk = np.ascontiguousarray(
        np.concatenate(
            [bq.reshape(NT, P).T, bk.reshape(NT, P).T], axis=1
        ).astype(np.float32)
    ) * WSCALE
    bv_row = bv.astype(np.float32).reshape(1, H)

    in_maps = []
    for b in range(B):
        hsT = np.ascontiguousarray(hidden_states[b].T.astype(np.float32))
        h8 = hsT.astype(FP8NP)
        d8 = (WSCALE * (hsT - h8.astype(np.float32))).astype(FP8NP)
        in_maps.append(
            {
                "h8p": pack_pair_rows(h8),
                "d8p": pack_pair_rows(d8),
                "wqk": wqk_,
                "wvab": wvab_,
                "bqk": bqk,
                "bv_row": bv_row,
            }
        )
    return in_maps


def kernel(hidden_states, Wq, bq, Wk, bk, Wv, bv):
    hidden_states = np.asarray(hidden_states, dtype=np.float32)
    Wq = np.asarray(Wq, dtype=np.float32)
    bq = np.asarray(bq, dtype=np.float32)
    Wk = np.asarray(Wk, dtype=np.float32)
    bk = np.asarray(bk, dtype=np.float32)
    Wv = np.asarray(Wv, dtype=np.float32)
    bv = np.asarray(bv, dtype=np.float32)

    B, S, H = hidden_states.shape
    NH = H // HD
    assert B == N_CORES, "one batch element per core"

    nc = _get_program(S, H, NH, N_CORES)
    in_maps = make_in_maps(hidden_states, Wq, bq, Wk, bk, Wv, bv)
    res = run_bass_kernel_spmd(nc, in_maps, core_ids=list(range(N_CORES)))
    out = np.empty((B, S, H), np.float32)
    for b in range(B):
        out[b] = res.results[b]["out"]
    return out


if __name__ == "__main__":
    build_program(1024, 1024, 16)
    print("build ok")


# revision 29
# speedup vs baseline: 1.0412x; 1.0412x over previous
"""Trainium2 Bass kernel for nn_CustomAttention (B=8, S=1024, H=1024, NH=16).

Strategy: data-parallel over batch — one batch element per NeuronCore, no
collectives. Host does layout-only prep (transposes / fp8+bf16 casts); all
FLOPs run on device.

v3: the Q/K/V projections run as fp8e4m3 DoubleRow matmuls (2 output
cols/cycle) with a 3-term RESIDUAL decomposition that keeps full accuracy:
  A  = fp8(32*W.T)              (weights in e4m3's sweet spot)
  B  = fp8(1024*(W.T - A/32))   (weight quantization residual)
  h8 = fp8(hs.T), d8 = fp8(32*(hs.T - h8))   (activation + residual)
  psA = h8@A + d8@A   (scale 32*q)     psB = h8@B   (scale 1024*q-residual)
  qt  = psA + psB/32 + 32*bq   (DVE: scalar_tensor_tensor + tensor_scalar_add)
Dropped terms are O(eps^2) ~ 0.1%, so logit noise stays ~0.002 (budget 0.015
— plain fp8's 0.04 logit noise fails the 2e-2 gate on concentrated softmax
rows). Projections cost 12 DR c-steps per 512-col chunk = 0.75x of bf16.
Scores / exp / ctx stay bf16 (exact-enough): fp8 anywhere in the softmax
value path was measured over-tolerance.

Scale bookkeeping: qt/kt = 32*(q+bq) bf16 -> scores psum = 1024*logits*8;
exp scale = (1/8)/1024; V' = 32*V bf16 with ones-col 32.0 so the DVE
reciprocal of den = 1/(32*Z) cancels everything in the normalize.

Per-core dataflow (as v1 otherwise):
  scoresT[s,l] per head = kt_h(stationary) . qt_h -> PSUM [128, S]
  expT = Exp(scores * scale) -> bf16 SBUF
  ctx[l,d] per (head, l): stationary expT chunk, moving V'[s, 65] (64 V cols
    + ones col) accumulated over s-tiles in one PSUM bank.
  normalize: DVE reciprocal + scalar_tensor_tensor (folds bv).
Schedule: software-pipelined sections; section t runs scores+exp(t), a ctx
work-queue (quota-paced so PE never outruns Act), proj(t+1) chunk-steps, and
V' chunk-steps in sections 0-2. PE is the critical engine (~144us busy);
Act's exp stream is ~133us; all steady-state DMAs stay off the Act queue.
PSUM: scores 2x2 banks, projA 1, projB 1, ctx 2x1 = 8 banks.
"""
import sys

sys.path.insert(0, "/opt/trn_rl_repo")

import numpy as np
import ml_dtypes
from collections import deque
from contextlib import ExitStack

from concourse import bacc, tile, mybir
from concourse.bass_utils import run_bass_kernel_spmd

F32 = mybir.dt.float32
BF16 = mybir.dt.bfloat16
FP8 = mybir.dt.float8e4
AF = mybir.ActivationFunctionType
ALU = mybir.AluOpType
DR = mybir.MatmulPerfMode.DoubleRow

P = 128
HD = 64
N_CORES = 8
WSCALE = 32.0
RESCALE = 1024.0  # weight-residual scale (WSCALE * WSCALE)
RINV = 1.0 / WSCALE

FP8NP = ml_dtypes.float8_e4m3


def _chunks(total, size=512):
    out = []
    a = 0
    while a < total:
        out.append((a, min(a + size, total)))
        a += size
    return out


def build_program(S, H, NH, num_devices=N_CORES):
    """One SPMD program; every core runs it on its own batch element."""
    KT = H // P          # h-tiles (contraction tiles)
    KP = KT // 2         # DoubleRow contraction pairs
    NT = H // P          # o-tiles
    ST = S // P          # s-tiles / l-tiles
    HPT = P // HD        # heads per o-tile (2)
    assert NH * HD == H and HPT == 2 and S == H
    SCALE = 1.0 / float(np.sqrt(HD))
    EXP_SCALE = SCALE / (WSCALE * WSCALE)

    nc = bacc.Bacc(
        "TRN2", target_bir_lowering=False, debug=False, num_devices=num_devices
    )

    # hsT pairs: row (c*P+p) = concat_i X[(2c+i)*P+p, :]
    h8p = nc.dram_tensor("h8p", [KP * P, 2 * S], FP8, kind="ExternalInput")
    d8p = nc.dram_tensor("d8p", [KP * P, 2 * S], FP8, kind="ExternalInput")
    # q/k weights DoubleRow-packed and group-merged per o-tile row block:
    # row (t*P+p), col (g*KP*2*P + c*2P + i*P + m), g in {qA,qB,kA,kB}
    wqk = nc.dram_tensor("wqk", [NT * P, 4 * KP * 2 * P], FP8,
                         kind="ExternalInput")
    # wv pairs (moving operand layout), A|B merged: col (s*2H + i*H + o)
    wvab = nc.dram_tensor("wvab", [KP * P, 2 * 2 * H], FP8,
                          kind="ExternalInput")
    bqk = nc.dram_tensor("bqk", [P, 2 * NT], F32, kind="ExternalInput")
    bv_row = nc.dram_tensor("bv_row", [1, H], F32, kind="ExternalInput")
    outD = nc.dram_tensor("out", [S, H], F32, kind="ExternalOutput")

    with tile.TileContext(nc) as tc, ExitStack() as ctx:
        consts = ctx.enter_context(tc.tile_pool(name="consts", bufs=1))
        hstp = ctx.enter_context(tc.tile_pool(name="hstp", bufs=2))
        wstr = ctx.enter_context(tc.tile_pool(name="wstr", bufs=3))
        qkp = ctx.enter_context(tc.tile_pool(name="qkp", bufs=4))
        tmpp = ctx.enter_context(tc.tile_pool(name="tmpp", bufs=2))
        vvp = ctx.enter_context(tc.tile_pool(name="vvp", bufs=ST))
        expp = ctx.enter_context(tc.tile_pool(name="expp", bufs=54))
        rpl = ctx.enter_context(tc.tile_pool(name="rpl", bufs=4))
        outp = ctx.enter_context(tc.tile_pool(name="outp", bufs=2))
        big = ctx.enter_context(tc.tile_pool(name="big", bufs=2, space="PSUM"))
        prA = ctx.enter_context(tc.tile_pool(name="prA", bufs=1, space="PSUM"))
        prB = ctx.enter_context(tc.tile_pool(name="prB", bufs=1, space="PSUM"))
        cxp = ctx.enter_context(tc.tile_pool(name="cxp", bufs=2, space="PSUM"))

        # ---- input DMA: w(0) and hsT tiles first (feed the PE asap).
        # Few LARGE transfers: each dma_start costs ~0.6us of HWDGE
        # descriptor-gen, which was the v3.1 startup bottleneck.
        w_t = {}  # t -> [P, 4, KP, 2, P] tile; g in {qA,qB,kA,kB}

        def load_w(t):
            if t >= NT or t in w_t:
                return
            w = wstr.tile([P, 4, KP, 2, P], FP8, tag="wstr", name=f"w{t}")
            nc.sync.dma_start(
                out=w[:],
                in_=wqk[t * P : (t + 1) * P, :].rearrange(
                    "p (g c i m) -> p g c i m", g=4, c=KP, i=2
                ),
            )
            w_t[t] = w

        load_w(0)

        # h8 tiles on the gpsimd DMA queue (25ns issue cost); d8 on the SP
        # queue after the first weights (the proj pass order A,B,d8 gives d8
        # until ~6us to land). NOTHING rides the Act queue before the first
        # exp: each dma_start costs its sequencer ~667ns, directly delaying
        # Act's pipeline. wvB DMAs are emitted mid-section-0 between exps.
        h8all = hstp.tile([P, KP, 2, S], FP8, tag="ht", name="h8all")
        nc.gpsimd.dma_start(
            out=h8all[:],
            in_=h8p[:, :].rearrange("(c p) (i l) -> p c i l", p=P, i=2),
        )
        h8 = [h8all[:, c] for c in range(KP)]

        # q/k biases: one small DMA, needed first at the proj(0) drain
        consts_bqk = consts.tile([P, 2 * NT], F32, tag="bqk")
        nc.sync.dma_start(out=consts_bqk[:], in_=bqk[:])

        d8all = hstp.tile([P, KP, 2, S], FP8, tag="ht", name="d8all")
        nc.sync.dma_start(
            out=d8all[:],
            in_=d8p[:, :].rearrange("(c p) (i l) -> p c i l", p=P, i=2),
        )
        d8 = [d8all[:, c] for c in range(KP)]

        # wv moving tiles (A|B merged, one transfer on the idle gpsimd
        # queue; first needed at section 0 j>=3, lands ~9us)
        wvpool = ctx.enter_context(tc.tile_pool(name="wvpool", bufs=1))
        wvt = wvpool.tile([P, KP, 2, 2, H], FP8, tag="wv", name="wvab")
        nc.gpsimd.dma_start(
            out=wvt[:],
            in_=wvab[:, :].rearrange("(c p) (s i o) -> p c s i o", p=P, s=2, i=2),
        )
        wva = [wvt[:, c, 0] for c in range(KP)]
        wvb = [wvt[:, c, 1] for c in range(KP)]

        load_w(1)

        # ---- bv broadcast (first needed at the first ctx normalize) ----
        bv_sb = consts.tile([1, H], F32, tag="bv")
        bvb = consts.tile([P, H], F32, tag="bvb")
        nc.sync.dma_start(out=bv_sb[:], in_=bv_row[:])
        nc.gpsimd.partition_broadcast(bvb[:], bv_sb[:])

        qt_t = {}
        kt_t = {}
        ex_t = {}  # t -> {(hh, j): exp tile}
        vv = []

        # ---- emission helpers ----
        def proj_steps(t, kbig=False):
            """Generator yielding per 4-matmul pass (3 passes per 512-col
            chunk, 12 per matrix-tile): Q then K of o-tile t. kbig: K's
            psums live in one big-pool tile (preamble only, so K need not
            wait on Q's psum drain)."""
            for gi, (mat, bcol, store, tag) in enumerate(
                (("q", t, qt_t, "qt"), ("k", NT + t, kt_t, "kt"))
            ):
                wa = w_t[t][:, 2 * gi]
                wb = w_t[t][:, 2 * gi + 1]
                ot = qkp.tile([P, S], BF16, tag=tag, name=f"{tag}{t}")
                store[t] = ot
                for (a, b) in _chunks(S):
                    if kbig and mat == "k":
                        ps = big.tile([P, S], F32, tag="big", name=f"kps{a}")
                        psA_ap, psB_ap = ps[:, 0:512], ps[:, 512:1024]
                    else:
                        psA_ap = prA.tile([P, 512], F32, tag="prA",
                                          name="pA")[:]
                        psB_ap = prB.tile([P, 512], F32, tag="prB",
                                          name="pB")[:]
                    # psA = h8@A (scale 32*q); psB = d8@A + h8@B (both scale
                    # 1024: d8 and B each carry a 32x residual boost).
                    for c in range(KP):
                        nc.tensor.matmul(
                            psA_ap, wa[:, c, :, :], h8[c][:, :, a:b],
                            start=(c == 0), stop=(c == KP - 1), perf_mode=DR,
                        )
                    yield
                    for c in range(KP):
                        nc.tensor.matmul(
                            psB_ap, wb[:, c, :, :], h8[c][:, :, a:b],
                            start=(c == 0), stop=False, perf_mode=DR,
                        )
                    yield
                    for c in range(KP):
                        nc.tensor.matmul(
                            psB_ap, wa[:, c, :, :], d8[c][:, :, a:b],
                            start=False, stop=(c == KP - 1), perf_mode=DR,
                        )
                    # ISA s2s2d2: two PSUM tensor srcs are illegal, so
                    # combine in two one-PSUM ops.
                    tmp = tmpp.tile([P, 512], BF16, tag="tmp", name="tmp")
                    nc.vector.tensor_scalar(
                        tmp[:], psB_ap, RINV, consts_bqk[:, bcol : bcol + 1],
                        ALU.mult, ALU.add,
                    )
                    nc.vector.tensor_tensor(ot[:, a:b], psA_ap, tmp[:],
                                            ALU.add)
                    yield
            while True:
                yield

        def vprime_steps():
            """Generator yielding one atomic 512-col chunk (12 DR matmuls +
            drain) per step; psums ride a big-pool tile (split A|B halves)
            so the interleave with scores can never deadlock the prA/prB
            rings. V' = 32*V in bf16; col 64 = 32.0 (the moving ones-column
            that produces softmax denominators)."""
            for m in range(ST):
                vt = vvp.tile([P, NH, 66], BF16, tag="vv", name=f"vv{m}")
                for ci, (a, b) in enumerate(_chunks(H)):
                    psA = cxp.tile([P, 512], F32, tag="cx", name=f"vA{m}_{ci}")[:]
                    psB = cxp.tile([P, 512], F32, tag="cx", name=f"vB{m}_{ci}")[:]
                    for c in range(KP):
                        nc.tensor.matmul(
                            psA, h8[c][:, :, m * P : (m + 1) * P],
                            wva[c][:, :, a:b],
                            start=(c == 0), stop=(c == KP - 1), perf_mode=DR,
                        )
                    for c in range(KP):
                        nc.tensor.matmul(
                            psB, d8[c][:, :, m * P : (m + 1) * P],
                            wva[c][:, :, a:b],
                            start=(c == 0), stop=False, perf_mode=DR,
                        )
                    for c in range(KP):
                        nc.tensor.matmul(
                            psB, h8[c][:, :, m * P : (m + 1) * P],
                            wvb[c][:, :, a:b],
                            start=False, stop=(c == KP - 1), perf_mode=DR,
                        )
                    h0 = a // HD
                    tmpv = tmpp.tile([P, 512], BF16, tag="tmp", name="tmpv")
                    nc.vector.tensor_scalar_mul(tmpv[:], psB, RINV)
                    nc.vector.tensor_tensor(
                        vt[:, h0 : h0 + 512 // HD, 0:64],
                        psA.rearrange("p (h d) -> p h d", d=HD),
                        tmpv[:].rearrange("p (h d) -> p h d", d=HD),
                        ALU.add,
                    )
                    if ci == 1:
                        nc.vector.memset(vt[:, :, 64:65], WSCALE)
                        vv.append(vt)
                    yield
            while True:
                yield

        def emit_scores_exp(t, j):
            exs = ex_t.setdefault(t, {})
            for hh in range(HPT):
                r0 = hh * HD
                sc = big.tile([P, S], F32, tag="big", name=f"sc{t}_{j}_{hh}")
                for (a, b) in _chunks(S):
                    nc.tensor.matmul(
                        sc[:, a:b],
                        kt_t[t][r0 : r0 + HD, j * P : (j + 1) * P],
                        qt_t[t][r0 : r0 + HD, a:b],
                        start=True, stop=True,
                        tile_position=(r0, 0),
                    )
                e = expp.tile([P, S], BF16, tag="ex", name=f"ex{t}_{j}_{hh}")
                nc.scalar.activation(e[:], sc[:], AF.Exp, scale=EXP_SCALE)
                exs[(hh, j)] = e

        def emit_ctx(t, l, ot, cx_pool=None, cx_tag="cx"):
            exs = ex_t[t]
            pool_ = cx_pool or cxp
            cx = pool_.tile([P, 512], F32, tag=cx_tag, name=f"cx{t}_{l}")
            for hh in range(HPT):
                h = HPT * t + hh
                o = hh * 65
                for j in range(ST):
                    nc.tensor.matmul(
                        cx[:, o : o + 65],
                        exs[(hh, j)][:, l * P : (l + 1) * P],
                        vv[j][:, h, 0:65],
                        start=(j == 0), stop=(j == ST - 1),
                    )
            rc = rpl.tile([P, HPT], F32, tag="rc", name=f"rc{t}_{l}")
            dens = cx[:, 0 : 2 * 65].rearrange("p (h x) -> p h x", x=65)[:, :, 64:65]
            nc.vector.reciprocal(rc[:].rearrange("p (h x) -> p h x", x=1), dens)
            for hh in range(HPT):
                nc.vector.scalar_tensor_tensor(
                    ot[:, l, hh * HD : (hh + 1) * HD],
                    cx[:, hh * 65 : hh * 65 + 64],
                    rc[:, hh : hh + 1],
                    bvb[:, (HPT * t + hh) * HD : (HPT * t + hh + 1) * HD],
                    ALU.mult,
                    ALU.add,
                )

        # ---- PE warm-up ----
        # The PE clock ramps to full rate only after ~3us of continuous
        # execution; real work can't start until the first weight/hs DMAs
        # land, so burn that wait on scratch matmuls.
        wu = consts.tile([P, 4 * P], BF16, tag="wu")
        nc.gpsimd.memset(wu[:], 0.0)
        wups = cxp.tile([P, 512], F32, tag="cx", name="wups")
        for i in range(14):
            nc.tensor.matmul(
                wups[:, 0:384], wu[:, 0:P], wu[:, P : P + 384],
                start=(i == 0), stop=(i == 13),
            )

        # preamble: proj(0) Q fully + K chunk0 (scores j<=3 only read kt
        # cols < 512, so K chunk1 smears into section 0). K's psums go to
        # the big pool so they need not wait on Q's chunk drains.
        p0 = proj_steps(0, kbig=True)
        for _ in range(9):
            next(p0)

        vgen = vprime_steps()
        nvp = 0  # V' pass-steps emitted so far

        ots = {}
        ctx_done = {}

        def ctx_unit(tc_, l, split_dma=False, cx_pool=None, cx_tag="cx"):
            """Emit one ctx+normalize unit; DMA the o-tile column when all
            ST units of tc_ have been emitted (split_dma: one DMA per l)."""
            if tc_ not in ots:
                ots[tc_] = outp.tile([P, ST, P], F32, tag="ou", name=f"ou{tc_}")
                ctx_done[tc_] = 0
            emit_ctx(tc_, l, ots[tc_], cx_pool=cx_pool, cx_tag=cx_tag)
            ctx_done[tc_] += 1
            if split_dma:
                if l % 2:  # DMA l-1..l as one transfer, alternating queues
                    eng = nc.gpsimd if l % 4 == 1 else nc.sync
                    eng.dma_start(
                        out=outD[
                            (l - 1) * P : (l + 1) * P,
                            tc_ * P : (tc_ + 1) * P,
                        ].rearrange("(l p) c -> p l c", p=P),
                        in_=ots[tc_][:, l - 1 : l + 1, :],
                    )
            elif ctx_done[tc_] == ST:
                nc.gpsimd.dma_start(
                    out=outD[:, tc_ * P : (tc_ + 1) * P].rearrange(
                        "(l p) c -> p l c", p=P
                    ),
                    in_=ots[tc_][:],
                )

        # ---- sections 0..NT-1 ----
        # scores/exp(t) pace Act; proj(t+1) smears 12 pass-steps/section;
        # V' smears its 48 pass-steps over sections 0-3; ctx units drain
        # from a quota-paced queue (tile t-1 units only at j>=3: its last
        # exps land ~2us into section t). Per-j PE work stays just above
        # Act's ~2.1us/j exp pace so neither engine stalls the psum ring.
        cqueue = deque()

        def vchunks(t, j):
            # 16 atomic V' chunks over sections 0-2
            if t == 0:
                return 1 if j >= 3 else 0
            if t == 1:
                return 1 if j <= 5 else 0
            if t == 2:
                return 1 if j <= 4 else 0
            return 0

        def cquota(t, j):
            # front-load each tile's units so its exp tiles free up before
            # the expp ring (bufs=56) wraps around to reuse them.
            if t == 2:
                return 1 if j >= 6 else 0
            if t == 3:
                return 2 if j <= 2 else 1
            if 4 <= t <= 6:
                return 1 + (1 if j >= 4 else 0)
            if t == 7:
                return 2
            return 0

        for t in range(NT):
            pgen = proj_steps(t + 1) if t + 1 < NT else None
            if t >= 1:
                cqueue.extend((t - 1, l) for l in range(ST))
            for j in range(ST):
                emit_scores_exp(t, j)
                if t == 0 and j == 0:
                    # K chunk1 of proj(0): must fully precede sc(0,1)'s big-
                    # pool alloc (its drain frees Kc1's slot in the ring).
                    for _ in range(3):
                        next(p0)
                for _ in range(vchunks(t, j)):
                    next(vgen)
                    nvp += 1
                npop = cquota(t, j)
                while npop > 0 and cqueue and (
                    cqueue[0][0] <= t - 2 or j >= 3 or (t == NT - 1 and j >= 2)
                ):
                    tc_, l = cqueue.popleft()
                    ctx_unit(tc_, l)
                    npop -= 1
                if pgen is not None:
                    next(pgen)
                    if j % 2 == 1:
                        next(pgen)
            load_w(t + 2)
        assert nvp == 2 * ST

        # tail: remaining ctx units (tile 7, plus any stragglers), rotating
        # the ctx PSUM through the now-idle proj/scores rings.
        cqueue.extend((NT - 1, l) for l in range(ST))
        tail_rot = [(cxp, "cx"), (prA, "prA"), (prB, "prB"), (big, "big"),
                    (cxp, "cx")]
        i = 0
        while cqueue:
            tc_, l = cqueue.popleft()
            pool_, tag_ = tail_rot[i % 5]
            i += 1
            ctx_unit(tc_, l, split_dma=(tc_ == NT - 1), cx_pool=pool_,
                     cx_tag=tag_)

    nc.compile()
    return nc


_CACHE = {}


def _get_program(S, H, NH, num_devices):
    key = (S, H, NH, num_devices)
    if key not in _CACHE:
        _CACHE[key] = build_program(S, H, NH, num_devices)
    return _CACHE[key]


def make_in_maps(hidden_states, Wq, bq, Wk, bk, Wv, bv):
    B, S, H = hidden_states.shape
    NT = H // P
    KT = H // P
    KP = KT // 2

    def pack_pair_rows(X):
        # [KT*P, W] -> [KP*P, 2W]: row c*P+p = concat_i X[(2c+i)*P+p, :]
        r = np.ascontiguousarray(X).reshape(KP, 2, P, -1)
        return np.ascontiguousarray(r.transpose(0, 2, 1, 3).reshape(KP * P, -1))

    def pack_w_dr(wT):
        # DoubleRow stationary pack: row (t*P+p), col (c*2P+i*P+m)
        #   = wT[(2c+i)*P+p, t*P+m]
        w5 = np.ascontiguousarray(wT).reshape(KP, 2, P, NT, P)
        return np.ascontiguousarray(
            w5.transpose(3, 2, 0, 1, 4).reshape(NT * P, KP * 2 * P)
        )

    def residual_pair(wT):
        A = (WSCALE * wT).astype(FP8NP)
        Bm = (RESCALE * (wT - A.astype(np.float32) / WSCALE)).astype(FP8NP)
        return A, Bm

    qA, qB = residual_pair(np.ascontiguousarray(Wq.T.astype(np.float32)))
    kA, kB = residual_pair(np.ascontiguousarray(Wk.T.astype(np.float32)))
    vA, vB = residual_pair(np.ascontiguousarray(Wv.T.astype(np.float32)))

    wqk_ = np.concatenate(
        [pack_w_dr(qA), pack_w_dr(qB), pack_w_dr(kA), pack_w_dr(kB)], axis=1
    )
    wvab_ = np.concatenate([pack_pair_rows(vA), pack_pair_rows(vB)], axis=1)

    bqk = np.ascontiguousarray(
        np.concatenate(
            [bq.reshape(NT, P).T, bk.reshape(NT, P).T], axis=1
        ).astype(np.float32)
    ) * WSCALE
    bv_row = bv.astype(np.float32).reshape(1, H)

    in_maps = []
    for b in range(B):
        hsT = np.ascontiguousarray(hidden_states[b].T.astype(np.float32))
        h8 = hsT.astype(FP8NP)
        d8 = (WSCALE * (hsT - h8.astype(np.float32))).astype(FP8NP)
        in_maps.append(
            {
                "h8p": pack_pair_rows(h8),
                "d8p": pack_pair_rows(d8),
                "wqk": wqk_,
                "wvab": wvab_,
                "bqk": bqk,
                "bv_row": bv_row,
            }
        )
    return in_maps


def kernel(hidden_states, Wq, bq, Wk, bk, Wv, bv):
    hidden_states = np.asarray(hidden_states, dtype=np.float32)
    Wq = np.asarray(Wq, dtype=np.float32)
    bq = np.asarray(bq, dtype=np.float32)
    Wk = np.asarray(Wk, dtype=np.float32)
    bk = np.asarray(bk, dtype=np.float32)
    Wv = np.asarray(Wv, dtype=np.float32)
    bv = np.asarray(bv, dtype=np.float32)

    B, S, H = hidden_states.shape
    NH = H // HD
    assert B == N_CORES, "one batch element per core"

    nc = _get_program(S, H, NH, N_CORES)
    in_maps = make_in_maps(hidden_states, Wq, bq, Wk, bk, Wv, bv)
    res = run_bass_kernel_spmd(nc, in_maps, core_ids=list(range(N_CORES)))
    out = np.empty((B, S, H), np.float32)
    for b in range(B):
        out[b] = res.results[b]["out"]
    return out


if __name__ == "__main__":
    build_program(1024, 1024, 16)
    print("build ok")
